# revision 6
# baseline (speedup 1.0000x reference)
"""EntityMultiAttnQMixer kernel — 8-way data-parallel over NeuronCores.

Shards the flattened batch*time dimension (BS = 64*256 = 16384) across the
8 cores; the ~100KB of params are replicated. The 4 hypernet branches
(w1/b1/w2/b2) are stacked and vmapped so each matmul is 4x larger, queries
are only computed for the A=16 agent slots actually used, and head
splitting avoids data transposes (einsum on [N,T,H,hd] directly).
"""

import numpy as np
import jax
import jax.numpy as jnp

B, S, T, E = 64, 256, 32, 64
D, H, A = 64, 4, 16
HD = D // H
NCORES = 8
BS = B * S
NETS = ('w1', 'b1', 'w2', 'b2')


def _shard_fn(agent_qs, entities, entity_mask, P):
    # agent_qs: [N, A]; entities: [N, T, E]; entity_mask: [N, T] bool
    # P: dict of per-net stacked params, leading axis 4 in NETS order.
    n = entities.shape[0]
    emask = entity_mask
    amask = emask[:, :A]                                   # [N, A]
    kmask = emask[:, None, None, :]                        # [N, 1, 1, T]
    qs = agent_qs[:, None, :]                              # [N, 1, A]

    def z_one(ew, eb, qwz, qbz, kw, kb, vwz, vbz, ow, ob, hw, hb):
        x = jax.nn.relu(entities @ ew + eb)                # [N, T, D]
        k = x @ kw + kb                                    # [N, T, D]
        # per-head weights are zero-embedded into full-D columns, so the
        # head split happens inside one [A*H, D] x [D, T] contraction
        qz = (x[:, :A] @ qwz + qbz).reshape(n, A, H, D)
        logits = jnp.einsum('bqhd,bkd->bhqk', qz, k) / jnp.sqrt(jnp.float32(HD))
        logits = jnp.where(kmask, -1e9, logits)
        attn = jax.nn.softmax(logits, axis=-1)
        vz = (x @ vwz + vbz).reshape(n, T, H, D)
        out = jnp.einsum('bhqk,bkhd->bqd', attn, vz)       # [N, A, D]
        y = out @ ow + ob                                  # [N, A, D]
        y = jnp.where(amask[:, :, None], 0.0, y)
        return y @ hw + hb                                 # [N, A, D]

    z_all = jax.vmap(z_one)(
        P['emb_w'], P['emb_b'], P['q_wz'], P['q_bz'], P['k_w'], P['k_b'],
        P['v_wz'], P['v_bz'], P['o_w'], P['o_b'], P['hyper_w'], P['hyper_b'])
    z_w1, z_b1, z_w2, z_b2 = (z_all[i] for i in range(4))

    w_1 = jax.nn.softmax(z_w1, axis=-1)                    # [N, A, D]
    b_1 = jnp.where(amask[:, :, None], 0.0, z_b1).mean(1, keepdims=True)
    h = jax.nn.elu(qs @ w_1 + b_1)                         # [N, 1, D]
    w_2 = jnp.where(amask[:, :, None], 0.0,
                    jax.nn.softmax(z_w2, axis=-1)).mean(1, keepdims=True)
    b_2 = jnp.where(amask[:, :, None], 0.0, z_b2).mean((1, 2), keepdims=True)
    q_tot = h @ w_2.transpose(0, 2, 1) + b_2               # [N, 1, 1]
    return q_tot[:, 0, :]                                  # [N, 1]


_pmapped = None


def _get_pmapped():
    global _pmapped
    if _pmapped is None:
        _pmapped = jax.pmap(_shard_fn, in_axes=(0, 0, 0, None),
                            devices=jax.devices()[:NCORES])
    return _pmapped


def _stack_params(params):
    f32 = lambda a: np.asarray(a, np.float32)
    P = {}
    P['emb_w'] = np.stack([f32(params[f'emb_{n}'][0]) for n in NETS])
    P['emb_b'] = np.stack([f32(params[f'emb_{n}'][1]) for n in NETS])
    for proj in ('k', 'o'):
        P[f'{proj}_w'] = np.stack([f32(params[f'attn_{n}'][proj][0]) for n in NETS])
        P[f'{proj}_b'] = np.stack([f32(params[f'attn_{n}'][proj][1]) for n in NETS])
    # q/v weights zero-embedded per head into full-D output columns:
    # [D, H*D] with head h's HD-wide slice landing at columns h*D+h*HD.
    for proj in ('q', 'v'):
        wz = np.zeros((len(NETS), D, H, D), np.float32)
        bz = np.zeros((len(NETS), H, D), np.float32)
        for i, n in enumerate(NETS):
            w, b = params[f'attn_{n}'][proj]
            w, b = f32(w), f32(b)
            for h in range(H):
                sl = slice(h * HD, (h + 1) * HD)
                wz[i, :, h, sl] = w[:, sl]
                bz[i, h, sl] = b[sl]
        P[f'{proj}_wz'] = wz.reshape(len(NETS), D, H * D)
        P[f'{proj}_bz'] = bz.reshape(len(NETS), H * D)
    P['hyper_w'] = np.stack([f32(params[f'hyper_{n}'][0]) for n in NETS])
    P['hyper_b'] = np.stack([f32(params[f'hyper_{n}'][1]) for n in NETS])
    return {k: jnp.asarray(v) for k, v in P.items()}


def kernel(agent_qs, entities, entity_mask, params):
    agent_qs = np.asarray(agent_qs, dtype=np.float32).reshape(NCORES, BS // NCORES, A)
    entities = np.asarray(entities, dtype=np.float32).reshape(NCORES, BS // NCORES, T, E)
    entity_mask = np.asarray(entity_mask).reshape(NCORES, BS // NCORES, T)
    P = _stack_params(params)
    out = _get_pmapped()(agent_qs, entities, entity_mask, P)
    out = np.asarray(out).reshape(B, S, 1).astype(np.float32)
    return out
